# revision 12
# baseline (speedup 1.0000x reference)
"""Trainium2 Bass kernel for nn_AudioDeviceModel (dense_cnn, memory-bound).

The reference model applies a chain of dilated kernel-size-2 convs to a
length-1 sequence with SAME padding.  For dilation d the two taps land at
padded positions 0 and d while the real sample sits at position d//2, so
every conv after the first reduces to its bias; the first conv (dilation 1,
pad_low=0) reduces to tap 0: a dot product of x[b, :] with w1[0, :, 0].
The whole model is therefore

    out[b, j] = (x[b, :] . w1[0, :, 0]) * wd[0, j] + bd_eff[j]
    bd_eff[j] = (b1 + b2 + b3 + b4 + b5) * wd[0, j] + bd[j]

(verified numerically against the jax reference to 1e-7).  This is a pure
memory-bound row-wise dot product over a 512 MiB matrix.

Strategy: data-parallel across 8 NeuronCores (1024 rows each).  Per core,
stream x in natural-layout [128, F] tiles (contiguous per-partition DMA at
full HBM bandwidth) and do multiply+reduce in a single DVE pass per tile
with scalar_tensor_tensor(op0=mult imm 1.0, op1=mult v, accum_out=partial).
The tiny epilogue (outer product with wd plus bias) is one fused
scalar_tensor_tensor per 128-row block.

This container's walrus build only accepts ONE on_wait and ONE on_update
per instruction, while Tile emits multi-wait instructions (kernel-tail
drain, multi-dependency compute ops).  legalize_bir_sync() splits the
extras into standalone EventSemaphore/NoOp instructions on the same engine
(sequencers are in-order, so a wait immediately before an instruction is
equivalent; trailing updates only on non-DMA instructions).
"""

import json

import numpy as np

import concourse.bass as bass
import concourse.mybir as mybir
import concourse.tile as tile
from concourse.bass_utils import run_bass_kernel_spmd

FP32 = mybir.dt.float32

N_CORES = 8
B_FULL = 8192
L = 16384
J = 128
B_CORE = B_FULL // N_CORES  # 1024
P = 128                     # SBUF partitions
F = 8192                    # L-chunk (free dim) per DVE op / DMA tile


def legalize_bir_sync(bir_bytes: bytes) -> bytes:
    """Split >1 on_wait / on_update per instruction for this walrus build."""
    mod = json.loads(bir_bytes)
    for fn in mod["functions"]:
        for bb in fn["blocks"]:
            out = []
            for ins in bb["instructions"]:
                si = ins.get("sync_info")
                waits = (si or {}).get("on_wait") or []
                ups = (si or {}).get("on_update") or []
                if len(waits) > 1:
                    for i, w in enumerate(waits[:-1]):
                        out.append({
                            "debug": ins.get("debug"),
                            "engine": ins["engine"],
                            "ins": [],
                            "outs": [],
                            "name": f"{ins['name']}_lw{i}",
                            "opcode": "EventSemaphore",
                            "sync_info": {"on_update": [], "on_wait": [w]},
                        })
                    si["on_wait"] = [waits[-1]]
                out.append(ins)
                if len(ups) > 1:
                    if ins.get("opcode") == "DMACopy":
                        raise RuntimeError(
                            f"multi-update on DMA {ins['name']} cannot be legalized"
                        )
                    for i, u in enumerate(ups[1:]):
                        out.append({
                            "debug": ins.get("debug"),
                            "engine": ins["engine"],
                            "ins": [],
                            "outs": [],
                            "name": f"{ins['name']}_lu{i}",
                            "opcode": "NoOp",
                            "sync_info": {"on_update": [u], "on_wait": []},
                        })
                    si["on_update"] = [ups[0]]
            bb["instructions"] = out
    return json.dumps(mod).encode()


def install_legalizer(nc):
    orig = nc.to_json_bytes

    def patched():
        return legalize_bir_sync(orig())

    nc.to_json_bytes = patched
    return nc


def build_module(b_core: int = B_CORE, l: int = L, f: int = F) -> bass.Bass:
    n_bb = b_core // P
    n_ch = l // f
    nc = bass.Bass()
    x_d = nc.dram_tensor("x", [b_core, l], FP32, kind="ExternalInput")
    v_d = nc.dram_tensor("v", [l], FP32, kind="ExternalInput")
    wd_d = nc.dram_tensor("wdrow", [J], FP32, kind="ExternalInput")
    bd_d = nc.dram_tensor("bdeff", [J], FP32, kind="ExternalInput")
    out_d = nc.dram_tensor("out", [b_core, J], FP32, kind="ExternalOutput")

    with tile.TileContext(nc) as tc:
        with (
            tc.tile_pool(name="consts", bufs=1) as consts,
            tc.tile_pool(name="xp", bufs=4) as xp,
            tc.tile_pool(name="accp", bufs=4) as accp,
            tc.tile_pool(name="outp", bufs=2) as outp,
        ):
            # v replicated across all 128 partitions via stride-0 DMA
            # broadcasts, one tile per L-chunk so each STT depends only on
            # its own chunk.  Chunk 0 is the FIRST DMA on the fast SP ring
            # (ready ~20us); x(c=0) tiles ride the ACT ring so the first
            # STT isn't queued behind it; chunk 1 loads on the gpsimd ring.
            v_bs = []
            for c in range(n_ch):
                v_bc = consts.tile([P, f], FP32, name=f"vb{c}", tag=f"vb{c}")
                src = v_d[c * f:(c + 1) * f].unsqueeze(0).partition_broadcast(P)
                (nc.sync if c == 0 else nc.gpsimd).dma_start(out=v_bc, in_=src)
                v_bs.append(v_bc)
            wd_b = consts.tile([P, J], FP32)
            nc.gpsimd.dma_start(out=wd_b, in_=wd_d[:].unsqueeze(0).partition_broadcast(P))
            bd_b = consts.tile([P, J], FP32)
            nc.gpsimd.dma_start(out=bd_b, in_=bd_d[:].unsqueeze(0).partition_broadcast(P))

            # x stream: chunk 0 of each row-block on the ACT HWDGE ring,
            # chunk 1 on the SP HWDGE ring — two rings stay deep so the 16
            # shared SDMA engines never drain between DMA instructions.
            dma_engines = (nc.scalar, nc.sync)
            for bb in range(n_bb):
                acc = accp.tile([P, n_ch], FP32)
                for c in range(n_ch):
                    x_t = xp.tile([P, f], FP32)
                    dma_engines[c % 2].dma_start(
                        out=x_t, in_=x_d[bb * P:(bb + 1) * P, c * f:(c + 1) * f]
                    )
                    # x_t = x_t * v (in place); acc[:, c] = sum over free dim
                    nc.vector.scalar_tensor_tensor(
                        out=x_t,
                        in0=x_t,
                        scalar=1.0,
                        in1=v_bs[c],
                        op0=mybir.AluOpType.mult,
                        op1=mybir.AluOpType.mult,
                        accum_out=acc[:, c:c + 1],
                    )
                t = accp.tile([P, 1], FP32, name=f"t{bb}", tag="t")
                nc.vector.tensor_reduce(
                    out=t, in_=acc, axis=mybir.AxisListType.X,
                    op=mybir.AluOpType.add,
                )
                # out[b, j] = wd[j] * t[b] + bd_eff[j]
                o_t = outp.tile([P, J], FP32)
                nc.vector.scalar_tensor_tensor(
                    out=o_t,
                    in0=wd_b,
                    scalar=t,
                    in1=bd_b,
                    op0=mybir.AluOpType.mult,
                    op1=mybir.AluOpType.add,
                )
                nc.sync.dma_start(out=out_d[bb * P:(bb + 1) * P, :], in_=o_t)
    install_legalizer(nc)
    return nc


_module_cache: dict = {}


def get_module() -> bass.Bass:
    if "nc" not in _module_cache:
        _module_cache["nc"] = build_module()
    return _module_cache["nc"]


def make_in_maps(inputs: dict) -> list[dict]:
    """Shard the full inputs into one input map per core (pure data parallel
    on the batch dim; tiny weights replicated)."""
    x = np.ascontiguousarray(np.asarray(inputs["x"], dtype=np.float32))
    w1 = np.asarray(inputs["w1"], dtype=np.float32)
    v = np.ascontiguousarray(w1[0, :, 0])
    s0 = float(sum(
        np.asarray(inputs[k], np.float32).reshape(-1)[0]
        for k in ("b1", "b2", "b3", "b4", "b5")
    ))
    wd_row = np.ascontiguousarray(np.asarray(inputs["wd"], np.float32)[0, :])
    bd = np.asarray(inputs["bd"], np.float32).reshape(-1)
    bd_eff = np.ascontiguousarray((s0 * wd_row + bd).astype(np.float32))

    return [
        {
            "x": np.ascontiguousarray(x[c * B_CORE:(c + 1) * B_CORE]),
            "v": v,
            "wdrow": wd_row,
            "bdeff": bd_eff,
        }
        for c in range(N_CORES)
    ]


def kernel(**inputs) -> np.ndarray:
    nc = get_module()
    in_maps = make_in_maps(inputs)
    res = run_bass_kernel_spmd(nc, in_maps, core_ids=list(range(N_CORES)))
    return np.concatenate([r["out"] for r in res.results], axis=0)
